# revision 50
# baseline (speedup 1.0000x reference)
"""Trainium2 Bass kernel for nn_Block_47193100648803.

Contract: kernel(**inputs) takes FULL unsharded inputs (numpy), returns the
FULL (N, O, T, V) output. Internally shards data-parallel over N across the
8 NeuronCores (one batch element per core, weights replicated).

Layout: channels on SBUF partitions (C=256 -> 2 half-tiles of 128), tokens on
the free axis; the temporal-window unfold is expressed with overlapping
strided access patterns (no data movement). All dense matmuls run bf16
(full PE rate); LN statistics come from PE ones-matmuls, rstd = Sqrt +
reciprocal_approx_fast, and normalization is applied AFTER the following
matmul ((x@Wg - mean*colsum(Wg))*rstd), so stats and main matmuls are
independent PE work. Wt@Wp is pre-fused; the pooled-attention "fronts"
(softmax via DRAM-bounce transpose DMAs, pooled q/k) are split into
latency-tolerant stages; ka logits use a Wka x pq outer-product so the k*pq
product is never materialized.

Schedule: phase 1 produces q/k/v/px per frame-tile with the first chunks'
fronts interleaved; phase 2 drives each chunk's 9-stage back pipeline
(att -> FFN-LN -> W1+gelu -> W2+residual -> temporal-LN -> c1+gelu -> c2)
as generators skewed 2 rounds apart, backs emitted before fronts in each
round so psum-consumer ops sit early in the engine FIFOs. Elementwise work
is spread across vector/gpsimd/scalar to keep the PE's in-order stream fed
(the HAM clock gate halves the PE clock after ~3.4us idle windows).
"""

import os
import sys

import numpy as np

for _p in ("/opt/trn_rl_repo", "/root/.axon_site/_ro/trn_rl_repo"):
    if os.path.isdir(_p) and _p not in sys.path:
        sys.path.append(_p)

import concourse.bass as bass
import concourse.tile as tile
from concourse import bacc, bass_utils, mybir
from concourse.masks import make_identity

f32 = mybir.dt.float32
f32r = mybir.dt.float32r
bf16 = mybir.dt.bfloat16
AF = mybir.ActivationFunctionType
ALU = mybir.AluOpType
AX = mybir.AxisListType

# ---- problem constants (hardcoded per spec) ----
N_CORES = 8
C, T, V = 256, 128, 25
H = 8
W = 3
O = 256
L = W * V                 # 75
FT = T + 2                # 130 padded frames
F = FT * V                # 3250 real frame columns (zero pads at both ends)
F_PAD = 3328              # allocated frame columns (8 * 416, fp32r-even subs)
G = T                     # 128 groups per core
GL = G * L                # 9600 group-stage columns
SCALE = 1.0 / (32.0 ** 0.5)
EPS = 1e-5

FSUB = 416                # phase-1 matmul column tile (even, 256..512)
N_FSUB = F_PAD // FSUB    # 8
CH_G = 16                 # groups per chunk in phase 2
N_CH = G // CH_G          # 8
CH = CH_G * L             # 1200
SUB_G = 4                 # groups per matmul sub-tile
SUB = SUB_G * L           # 300
N_SUB = CH_G // SUB_G     # 4
SUBW = 400                # wide matmul sub (fp32r-even, 256..512)
N_SUBW = CH // SUBW       # 3
# bank-aligned dst slices for chunk-wide [128, CH] psum accumulation
BANK_SUBS = [(0, 512), (512, 512), (1024, 176)]


def _r(ap):
    return ap.bitcast(f32r)


def _view(t, offset, dims):
    """AP view on tile t: partition dim kept, free dims replaced."""
    return bass.AP(tensor=t.tensor, offset=t.offset + offset, ap=[t.ap[0]] + dims)


def unf(t, g0, gc):
    """Overlapping window view [128, gc, W, V] on a [128, F] frame tile."""
    return _view(t, g0 * V, [[V, gc], [V, W], [1, V]])


def seg(t, g0, gc):
    """[128, gc, L] view on a [128, GL] or chunk tile starting at group g0
    (g0 relative to tile origin)."""
    return _view(t, g0 * L, [[L, gc], [1, L]])


def bc_g(t, g0, gc):
    """Broadcast per-(c,g) [128, G] tile over L -> [128, gc, L] (step-0)."""
    return _view(t, g0, [[1, gc], [0, L]])


def build(nc):
    x_d = nc.dram_tensor("x", [C, T, V], f32, kind="ExternalInput").ap()
    wd = {}
    for nm in ["Wq", "Wk", "Wv", "Wt", "Wp", "W1", "W2", "c1_w"]:
        wd[nm] = nc.dram_tensor(nm, [C, C], f32, kind="ExternalInput").ap()
    wd["Wqa"] = nc.dram_tensor("Wqa", [C, H], f32, kind="ExternalInput").ap()
    wd["Wka"] = nc.dram_tensor("Wka", [C, H], f32, kind="ExternalInput").ap()
    wd["c2_w"] = nc.dram_tensor("c2_w", [W, C, O], f32, kind="ExternalInput").ap()
    bnames = ["ln1_g", "ln1_b", "bq", "bk", "bv", "bt", "bp", "ffn_g", "ffn_b",
              "b1", "b2", "tn_g", "tn_b", "c1_b", "c2_b"]
    for nm in bnames:
        wd[nm] = nc.dram_tensor(nm, [C], f32, kind="ExternalInput").ap()
    wd["bqa"] = nc.dram_tensor("bqa", [H], f32, kind="ExternalInput").ap()
    wd["bka"] = nc.dram_tensor("bka", [H], f32, kind="ExternalInput").ap()
    out_d = nc.dram_tensor("out", [O, T, V], f32, kind="ExternalOutput").ap()

    dbg = os.environ.get("BASS_DEBUG_SCRATCH") == "1"
    skind = "ExternalOutput" if dbg else "Internal"
    qa_d = nc.dram_tensor("qa_scr", [H, F_PAD], f32, kind=skind).ap()
    qw_d = nc.dram_tensor("qw_scr", [H, GL], bf16, kind=skind).ap()
    ka_d = nc.dram_tensor("ka_scr", [H, GL], f32, kind=skind).ap()
    kw_d = nc.dram_tensor("kw_scr", [H, GL], bf16, kind=skind).ap()
    row_d = nc.dram_tensor("row_scr", [3, C], f32).ap()

    with tile.TileContext(nc) as tc:
        with (
            tc.tile_pool(name="consts", bufs=1) as cp,
            tc.tile_pool(name="data", bufs=1) as dp,
        ):
            # front pools span both phases; entered before p1x for LIFO exit
            frp_cm = tc.tile_pool(name="front_sb", bufs=1)
            frp = frp_cm.__enter__()
            fps_cm = tc.tile_pool(name="front_ps", bufs=2, space="PSUM")
            fps = fps_cm.__enter__()

            # ---------- input load first (weights go on other DMA queues) ----
            p1x_cm = tc.tile_pool(name="p1_x", bufs=1)
            p1x = p1x_cm.__enter__()
            x_f = [p1x.tile([128, F_PAD], f32, tag=f"x_f{hh}", name=f"x_f{hh}")
                   for hh in range(2)]
            for hh in range(2):
                for qt in range(4):
                    ts0, ts1 = qt * (T // 4), (qt + 1) * (T // 4)
                    nc.gpsimd.dma_start(
                        out=_r(x_f[hh][:, V + ts0 * V:V + ts1 * V]),
                        in_=_r(x_d[hh * 128:(hh + 1) * 128, ts0:ts1, :]))
                nc.vector.memset(x_f[hh][:, 0:V], 0.0)
                nc.vector.memset(x_f[hh][:, F - V:F_PAD], 0.0)

            # ---------- weights / constants ----------
            # f32 masters live in a staging pool freed after setup; only the
            # bf16 working copies persist.
            wsp_cm = tc.tile_pool(name="wstage", bufs=1)
            wsp = wsp_cm.__enter__()
            wt = {}
            for nm in ["Wt", "Wp", "Wq", "Wk", "Wv", "W1", "W2", "c1_w"]:
                wt[nm] = [wsp.tile([128, C], f32, tag=f"w_{nm}{kh}", name=f"w_{nm}{kh}")
                          for kh in range(2)]
                for kh in range(2):
                    nc.sync.dma_start(out=_r(wt[nm][kh]),
                                      in_=_r(wd[nm][kh * 128:(kh + 1) * 128, :]))
            for nm in ["Wqa", "Wka"]:
                wt[nm] = [wsp.tile([128, H], f32, tag=f"w_{nm}{kh}", name=f"w_{nm}{kh}")
                          for kh in range(2)]
                for kh in range(2):
                    nc.sync.dma_start(out=_r(wt[nm][kh]),
                                      in_=_r(wd[nm][kh * 128:(kh + 1) * 128, :]))
            c2t = []
            for w in range(W):
                c2t.append([wsp.tile([128, O], f32, tag=f"w_c2_{w}{kh}", name=f"w_c2_{w}{kh}")
                            for kh in range(2)])
                for kh in range(2):
                    nc.sync.dma_start(out=_r(c2t[w][kh]),
                                      in_=_r(wd["c2_w"][w, kh * 128:(kh + 1) * 128, :]))

            def load_bias_col(nm):
                t = cp.tile([128, 2], f32, tag=f"b_{nm}", name=f"b_{nm}")
                src = bass.AP(tensor=wd[nm].tensor, offset=wd[nm].offset,
                              ap=[[1, 128], [128, 2]])
                nc.gpsimd.dma_start(out=t, in_=src)
                return t

            bias = {nm: load_bias_col(nm) for nm in bnames}
            for nm in ["bqa", "bka"]:
                t = cp.tile([H, 1], f32, tag=f"b_{nm}", name=f"b_{nm}")
                nc.sync.dma_start(out=t, in_=wd[nm])
                bias[nm] = t

            eps_t = cp.tile([128, 1], f32, tag="eps", name="eps_t")
            nc.vector.memset(eps_t, EPS)

            def fill_r(t, value):
                # constant fill with an f32r-typed output (plain Memset cannot
                # emit f32r): Copy(in*0 + value) ignores the uninitialized in_
                nc.scalar.activation(out=_r(t), in_=_r(t), func=AF.Copy,
                                     bias=float(value), scale=0.0)

            onesC = cp.tile([128, 128], f32, tag="onesC", name="onesC")
            fill_r(onesC, 1.0 / C)
            onesC_b = cp.tile([128, 128], bf16, tag="onesC_b", name="onesC_b")
            nc.scalar.activation(out=onesC_b, in_=onesC, func=AF.Copy)
            w2b = [cp.tile([128, C], bf16, tag=f"w2b{kh}", name=f"w2b{kh}") for kh in range(2)]
            wkab = [cp.tile([128, H], bf16, tag=f"wkab{kh}", name=f"wkab{kh}") for kh in range(2)]
            negones = cp.tile([128, 1], f32, tag="negones", name="negones")
            fill_r(negones, -1.0)
            negones_b = cp.tile([128, 1], bf16, tag="negones_b", name="negones_b")
            nc.scalar.activation(out=negones_b, in_=negones, func=AF.Copy)

            # folded weights: W1g = ffn_g*W1, c1g = tn_g*c1_w
            w1g = [cp.tile([128, C], bf16, tag=f"w1g{kh}", name=f"w1g{kh}") for kh in range(2)]
            c1g = [cp.tile([128, C], bf16, tag=f"c1g{kh}", name=f"c1g{kh}") for kh in range(2)]
            for kh in range(2):
                nc.vector.tensor_scalar_mul(w1g[kh], wt["W1"][kh], bias["ffn_g"][:, kh:kh + 1])
                nc.vector.tensor_scalar_mul(c1g[kh], wt["c1_w"][kh], bias["tn_g"][:, kh:kh + 1])
                nc.vector.tensor_copy(w2b[kh], wt["W2"][kh])
                nc.vector.tensor_copy(wkab[kh], wt["Wka"][kh])

            # bf16 copies of the phase-1 matmul weights (full-rate PE)
            wb = {}
            for nm in ["Wq", "Wk", "Wv", "Wp"]:
                wb[nm] = [cp.tile([128, C], bf16, tag=f"wb_{nm}{kh}", name=f"wb_{nm}{kh}")
                          for kh in range(2)]
                for kh in range(2):
                    nc.vector.tensor_copy(wb[nm][kh], wt[nm][kh])
            wqab = [cp.tile([128, H], bf16, tag=f"wqab{kh}", name=f"wqab{kh}") for kh in range(2)]
            for kh in range(2):
                nc.vector.tensor_copy(wqab[kh], wt["Wqa"][kh])
            # Wka replicated over the chunk's groups, g-major: [c, g, h]
            wkarep_f = [cp.tile([128, CH_G * H], f32, tag=f"wkarep_f{kh}",
                                name=f"wkarep_f{kh}") for kh in range(2)]
            wkarep = [cp.tile([128, CH_G * H], bf16, tag=f"wkarep{kh}",
                              name=f"wkarep{kh}") for kh in range(2)]
            for kh in range(2):
                srcr = bass.AP(tensor=wd["Wka"].tensor,
                               offset=wd["Wka"].offset + kh * 128 * H,
                               ap=[[H, 128], [0, CH_G], [1, H]])
                nc.sync.dma_start(out=_view(wkarep_f[kh], 0, [[H, CH_G], [1, H]]),
                                  in_=srcr)
                nc.vector.tensor_copy(wkarep[kh], wkarep_f[kh])
            c2b = []
            for w in range(W):
                c2b.append([cp.tile([128, O], bf16, tag=f"wb_c2_{w}{kh}", name=f"wb_c2_{w}{kh}")
                            for kh in range(2)])
                for kh in range(2):
                    nc.vector.tensor_copy(c2b[w][kh], c2t[w][kh])

            wtp = [cp.tile([128, C], bf16, tag=f"wtp{kh}", name=f"wtp{kh}") for kh in range(2)]
            negg = [cp.tile([1, C], bf16, tag=f"negg{i}", name=f"negg{i}")
                    for i in range(2)]  # [-G1], [-Gc1]

            # ---------- setup-scoped: Wtp = Wt@Wp, bias rows ----------
            with (
                tc.tile_pool(name="setup_sb", bufs=1) as sp,
                tc.tile_pool(name="setup_ps", bufs=2, space="PSUM") as spp,
            ):
                wtw = wt["Wt"]
                ident = sp.tile([128, 128], f32, tag="ident", name="ident")
                make_identity(nc, ident)

                for kh in range(2):
                    pacc = spp.tile([128, C], f32, tag="wtp_acc", name="pacc")
                    for mh in range(2):
                        ptr = spp.tile([128, 128], f32, tag="tr", name="ptr")
                        nc.tensor.transpose(ptr, wtw[kh][:, mh * 128:(mh + 1) * 128], ident)
                        a_t = sp.tile([128, 128], f32, tag="a_t", name="a_t")
                        nc.scalar.activation(out=_r(a_t), in_=ptr, func=AF.Copy)
                        nc.tensor.matmul(pacc, _r(a_t), _r(wt["Wp"][mh]),
                                         start=(mh == 0), stop=(mh == 1))
                    nc.scalar.activation(out=wtp[kh], in_=pacc, func=AF.Copy)

                def colvec(nm, kh):
                    t = sp.tile([128, 1], f32, tag=f"cv_{nm}{kh}", name=f"cv_{nm}{kh}")
                    src = bass.AP(tensor=wd[nm].tensor, offset=wd[nm].offset + kh * 128,
                                  ap=[[1, 128], [128, 1]])
                    nc.sync.dma_start(out=_r(t), in_=_r(src))
                    return t

                def rowvec(nm):
                    t = sp.tile([1, C], f32, tag=f"rv_{nm}", name=f"rv_{nm}")
                    nc.sync.dma_start(out=t, in_=wd[nm])
                    return t

                for i, (bnm, wmat, addnm) in enumerate([
                    ("bt", wt["Wp"], "bp"),
                    ("ffn_b", wt["W1"], "b1"),
                    ("tn_b", wt["c1_w"], "c1_b"),
                ]):
                    pr = spp.tile([1, C], f32, tag="rowacc", name="pr")
                    for kh in range(2):
                        nc.tensor.matmul(pr, _r(colvec(bnm, kh)), _r(wmat[kh]),
                                         start=(kh == 0), stop=(kh == 1))
                    row_i = sp.tile([1, C], f32, tag=f"row_i{i}", name=f"row_i{i}")
                    nc.vector.tensor_add(row_i, pr, rowvec(addnm))
                    nc.sync.dma_start(out=row_d[i:i + 1, :], in_=row_i)

                for i, wmat in enumerate([w1g, c1g]):
                    pg = spp.tile([1, C], f32, tag="rowacc", name="pg")
                    for kh in range(2):
                        nc.tensor.matmul(pg, negones_b, wmat[kh],
                                         start=(kh == 0), stop=(kh == 1))
                    nc.scalar.activation(out=negg[i], in_=pg, func=AF.Copy)

            wsp_cm.__exit__(None, None, None)

            # bounce bias rows back into per-partition [128, 2] layout
            btp_t = cp.tile([128, 2], f32, tag="btp", name="btp_t")
            B1_t = cp.tile([128, 2], f32, tag="B1", name="B1_t")
            Bc1_t = cp.tile([128, 2], f32, tag="Bc1", name="Bc1_t")
            for i, t in enumerate([btp_t, B1_t, Bc1_t]):
                src = bass.AP(tensor=row_d.tensor, offset=row_d.offset + i * C,
                              ap=[[1, 128], [128, 1]])
                nc.sync.dma_start(out=t[:, 0:1], in_=src)
                src2 = bass.AP(tensor=row_d.tensor, offset=row_d.offset + i * C + 128,
                               ap=[[1, 128], [128, 1]])
                nc.sync.dma_start(out=t[:, 1:2], in_=src2)

            # ---------- persistent activations ----------
            px_f = [dp.tile([128, F_PAD], bf16, tag=f"px_f{hh}", name=f"px_f{hh}") for hh in range(2)]
            q_f = [dp.tile([128, F_PAD], bf16, tag=f"q_f{hh}", name=f"q_f{hh}") for hh in range(2)]
            k_f = [dp.tile([128, F_PAD], bf16, tag=f"k_f{hh}", name=f"k_f{hh}") for hh in range(2)]
            v_f = [dp.tile([128, F_PAD], bf16, tag=f"v_f{hh}", name=f"v_f{hh}") for hh in range(2)]
            pq_b = [dp.tile([128, G], bf16, tag=f"pqb{hh}", name=f"pqb{hh}") for hh in range(2)]
            pk_b = [dp.tile([128, G], bf16, tag=f"pkb{hh}", name=f"pkb{hh}") for hh in range(2)]

            # ---------- chunk fronts (outer scope: used in both phases) ------

            def softmax_chunk(src_gather_ap, dst_dram, g0, tagp):
                """Per-chunk softmax in [128 = 16 groups x 8 heads, L] layout;
                writes normalized weights to dst_dram[h, cols]. Logits are
                tiny (|x|*scale << 1) so no max-subtraction is needed; the
                elementwise ops run on gpsimd to keep latency off the busy
                vector queue."""
                ag = frp.tile([128, L], f32, tag="sm_ag", bufs=4,
                              name=f"ag_{tagp}")
                nc.gpsimd.dma_start(out=ag, in_=src_gather_ap)
                e = frp.tile([128, L], f32, tag="sm_e", bufs=4,
                             name=f"e_{tagp}")
                nc.scalar.activation(out=e, in_=ag, func=AF.Exp, scale=SCALE)
                sm = frp.tile([128, 1], f32, tag="sm_s", bufs=4,
                              name=f"sm_{tagp}")
                nc.vector.reduce_sum(sm, e, axis=AX.X)
                rs = frp.tile([128, 1], f32, tag="sm_rs", bufs=4,
                              name=f"rs_{tagp}")
                nc.vector.reciprocal(rs, sm)
                wgn = frp.tile([128, L], bf16, tag="sm_w", bufs=4,
                               name=f"wgn_{tagp}")
                nc.gpsimd.tensor_scalar_mul(wgn, e, rs[:, 0:1])
                dst = bass.AP(tensor=dst_dram.tensor,
                              offset=dst_dram.offset + g0 * L,
                              ap=[[L, CH_G], [GL, H], [1, L]])
                nc.gpsimd.dma_start(out=dst, in_=wgn)

            def head_bcast(src_dram, g0, hh, tagp):
                """[128, CH] tile with partition c reading
                src_dram[c // 32 (+4*hh), chunk cols] via broadcast DMA."""
                t = frp.tile([128, CH], bf16, tag="bc", bufs=2,
                             name=f"bc_{tagp}")
                src = bass.AP(
                    tensor=src_dram.tensor,
                    offset=src_dram.offset + (hh * 4) * GL + g0 * L,
                    ap=[[GL, 4], [0, 32], [1, CH]])
                nc.sync.dma_start(out=t, in_=src)
                return t

            kp_store = {}
            z_store = {}

            def front_1a(cc):
                """qw softmax -> pooled query -> kp = k*pq (no PE work)."""
                g0 = cc * CH_G
                qa_gather = bass.AP(
                    tensor=qa_d.tensor, offset=qa_d.offset + g0 * V,
                    ap=[[V, CH_G], [F_PAD, H], [V, W], [1, V]])
                softmax_chunk(qa_gather, qw_d, g0, f"q{cc}")
                kp = []
                for hh in range(2):
                    qb = head_bcast(qw_d, g0, hh, f"q{cc}{hh}")
                    prod = frp.tile([128, CH], bf16, tag="prod", bufs=2,
                                    name="prod")
                    nc.gpsimd.tensor_mul(_view(prod, 0, [[L, CH_G], [1, L]]),
                                         unf(q_f[hh], g0, CH_G),
                                         _view(qb, 0, [[L, CH_G], [1, L]]))
                    with nc.allow_low_precision(reason="DVE reduce accumulates fp32; bf16 rounding only at output"):
                        nc.vector.reduce_sum(pq_b[hh][:, g0:g0 + CH_G],
                                             _view(prod, 0, [[L, CH_G], [1, L]]),
                                             axis=AX.X)
                    # kwc[c, g, h] = Wka[c,h] * pq[c,g]: tiny outer product;
                    # ka then contracts k-frames directly (no k*pq materialize)
                    kwc = frp.tile([128, CH_G * H], bf16, tag="kwc", bufs=2,
                                   name="kwc")
                    nc.vector.scalar_tensor_tensor(
                        out=kwc, in0=wkarep[hh], scalar=1.0,
                        in1=_view(pq_b[hh], g0, [[1, CH_G], [0, H]]),
                        op0=ALU.mult, op1=ALU.mult)
                    kp.append(kwc)
                kp_store[cc] = kp

            def front_1b(cc):
                """ka = kp @ Wka -> DRAM (PE work, one round after F1a)."""
                g0 = cc * CH_G
                col0 = g0 * L
                kwc = kp_store.pop(cc)
                ka_c = frp.tile([H, CH], f32, tag="ka_c", bufs=2, name="ka_c")
                for su in range(4):
                    pka = fps.tile([H, SUB], f32, tag="stat", name="pka")
                    for gi in range(SUB_G):
                        g = su * SUB_G + gi
                        ls = slice(gi * L, (gi + 1) * L)
                        for kh in range(2):
                            nc.tensor.matmul(
                                pka[:, ls],
                                _view(kwc[kh], g * H, [[1, H]]),
                                k_f[kh][:, (g0 + g) * V:(g0 + g) * V + L],
                                start=(kh == 0), stop=(kh == 1))
                    nc.scalar.activation(out=ka_c[:, su * SUB:(su + 1) * SUB],
                                         in_=pka, func=AF.Identity, bias=bias["bka"])
                nc.gpsimd.dma_start(out=ka_d[:, col0:col0 + CH], in_=ka_c)

            def front_2(cc):
                """kw softmax -> pooled key -> z = v * pk."""
                g0 = cc * CH_G
                col0 = g0 * L
                ka_gather = bass.AP(
                    tensor=ka_d.tensor, offset=ka_d.offset + col0,
                    ap=[[L, CH_G], [GL, H], [1, L]])
                softmax_chunk(ka_gather, kw_d, g0, f"k{cc}")
                z = []
                for hh in range(2):
                    kb = head_bcast(kw_d, g0, hh, f"k{cc}{hh}")
                    prod = frp.tile([128, CH], bf16, tag="prod", bufs=2,
                                    name="prod2")
                    nc.gpsimd.tensor_mul(_view(prod, 0, [[L, CH_G], [1, L]]),
                                         unf(k_f[hh], g0, CH_G),
                                         _view(kb, 0, [[L, CH_G], [1, L]]))
                    with nc.allow_low_precision(reason="DVE reduce accumulates fp32; bf16 rounding only at output"):
                        nc.vector.reduce_sum(pk_b[hh][:, g0:g0 + CH_G],
                                             _view(prod, 0, [[L, CH_G], [1, L]]),
                                             axis=AX.X)
                    zh = frp.tile([128, CH], bf16, tag="z", bufs=6,
                                  name=f"z{cc}_{hh}")
                    nc.gpsimd.tensor_mul(_view(zh, 0, [[L, CH_G], [1, L]]),
                                         unf(v_f[hh], g0, CH_G),
                                         bc_g(pk_b[hh], g0, CH_G))
                    z.append(zh)
                z_store[cc] = z

            # fronts for chunks 0/1 run in the phase-1 tail; later fronts are
            # scheduled inside the phase-2 driver on PE-quiet rounds
            P1_FRONT = {2: [lambda: front_1a(0)], 3: [lambda: front_1b(0)],
                        4: [lambda: front_2(0)], 5: [lambda: front_1a(1)],
                        6: [lambda: front_1b(1)], 7: [lambda: front_2(1)]}

            # ---------- phase 1: per-frame pipeline + chunk fronts ----------
            with (
                tc.tile_pool(name="p1_sb", bufs=2) as p1,
                tc.tile_pool(name="p1_ps", bufs=1, space="PSUM") as pp1,
                tc.tile_pool(name="p1_mm", bufs=3, space="PSUM") as pp1m,
            ):
                for s in range(N_FSUB):
                    sl = slice(s * FSUB, (s + 1) * FSUB)
                    x2 = [p1.tile([128, FSUB], bf16, tag=f"x2_{hh}", name=f"x2_{hh}")
                          for hh in range(2)]
                    for hh in range(2):
                        nc.vector.scalar_tensor_tensor(
                            out=x2[hh], in0=x_f[hh][:, sl], scalar=1.0,
                            in1=x_f[hh][:, sl], op0=ALU.mult, op1=ALU.mult)
                    pmean = pp1.tile([128, FSUB], f32, tag="pmean", name="pmean")
                    pmsq = pp1.tile([128, FSUB], f32, tag="pmsq", name="pmsq")
                    for hh in range(2):
                        nc.tensor.matmul(pmean, _r(onesC), _r(x_f[hh][:, sl]),
                                         start=(hh == 0), stop=(hh == 1))
                    for hh in range(2):
                        nc.tensor.matmul(pmsq, onesC_b, x2[hh],
                                         start=(hh == 0), stop=(hh == 1))
                    m2 = p1.tile([128, FSUB], f32, tag="m2", name="m2")
                    nc.scalar.activation(out=m2, in_=pmean, func=AF.Square)
                    var = p1.tile([128, FSUB], f32, tag="var", name="var")
                    nc.vector.tensor_sub(var, pmsq, m2)
                    srt = p1.tile([128, FSUB], f32, tag="sd", name="srt")
                    nc.scalar.activation(out=srt, in_=var, func=AF.Sqrt, bias=eps_t)
                    rstd = p1.tile([128, FSUB], f32, tag="rstd", name="rstd")
                    nc.vector.reciprocal_approx_fast(out=rstd, in_=srt)
                    nx = []
                    for hh in range(2):
                        xc = p1.tile([128, FSUB], f32, tag=f"xc{hh}", name=f"xc{hh}")
                        nc.vector.tensor_sub(xc, x_f[hh][:, sl], pmean)
                        xg = p1.tile([128, FSUB], f32, tag=f"xg{hh}", name=f"xg{hh}")
                        nc.vector.scalar_tensor_tensor(
                            out=xg, in0=xc, scalar=bias["ln1_g"][:, hh:hh + 1],
                            in1=rstd, op0=ALU.mult, op1=ALU.mult)
                        nxh = p1.tile([128, FSUB], bf16, tag=f"nx{hh}", name=f"nx{hh}")
                        nc.vector.tensor_scalar_add(nxh, xg, bias["ln1_b"][:, hh:hh + 1])
                        nx.append(nxh)
                    for mh in range(2):
                        pq_ = pp1m.tile([128, FSUB], f32, tag="mm", name="pq_")
                        for kh in range(2):
                            nc.tensor.matmul(pq_, wb["Wq"][kh][:, mh * 128:(mh + 1) * 128],
                                             nx[kh], start=(kh == 0), stop=(kh == 1))
                        nc.scalar.activation(out=q_f[mh][:, sl], in_=pq_, func=AF.Identity,
                                             bias=bias["bq"][:, mh:mh + 1])
                    for nm, bnm, dst in [("Wk", "bk", k_f), ("Wv", "bv", v_f)]:
                        for mh in range(2):
                            pm_ = pp1m.tile([128, FSUB], f32, tag="mm", name="pm_")
                            for kh in range(2):
                                nc.tensor.matmul(pm_,
                                                 wb[nm][kh][:, mh * 128:(mh + 1) * 128],
                                                 nx[kh], start=(kh == 0), stop=(kh == 1))
                            nc.scalar.activation(out=dst[mh][:, sl], in_=pm_,
                                                 func=AF.Identity,
                                                 bias=bias[bnm][:, mh:mh + 1])
                    pqa = pp1.tile([H, FSUB], f32, tag="pqa", name="pqa")
                    for kh in range(2):
                        nc.tensor.matmul(pqa, wqab[kh], nx[kh],
                                         start=(kh == 0), stop=(kh == 1))
                    qa_s = p1.tile([H, FSUB], f32, tag="qa_s", name="qa_s")
                    nc.scalar.activation(out=qa_s, in_=pqa, func=AF.Identity,
                                         bias=bias["bqa"])
                    nc.sync.dma_start(out=qa_d[:, sl], in_=qa_s)
                    # px = q@Wp + btp + x   (pre-added residual path for attn)
                    for mh in range(2):
                        pp_ = pp1m.tile([128, FSUB], f32, tag="mm", name="pp_")
                        for kh in range(2):
                            nc.tensor.matmul(pp_, wb["Wp"][kh][:, mh * 128:(mh + 1) * 128],
                                             q_f[kh][:, sl], start=(kh == 0), stop=(kh == 1))
                        nc.vector.scalar_tensor_tensor(
                            out=px_f[mh][:, sl], in0=pp_, scalar=btp_t[:, mh:mh + 1],
                            in1=x_f[mh][:, sl], op0=ALU.add, op1=ALU.add)
                    for fn in P1_FRONT.get(s, []):
                        fn()
            p1x_cm.__exit__(None, None, None)

            # ---------- phase 2: skewed back pipeline ----------
            # Each chunk's back half is a 9-stage generator; chunks are driven
            # with a 4-stage skew so (stats, stats) and (gelu, gelu) stages of
            # neighboring chunks are adjacent in the scalar stream (activation
            # table sets batch: sqrt-set then gelu-set, 2 loads per chunk) and
            # the PE always has an independent matmul stream to fill stalls.
            with (
                tc.tile_pool(name="p2_sb", bufs=1) as p2,
                tc.tile_pool(name="p2_ps", bufs=2, space="PSUM") as pmm,
            ):
                pst = fps
                def layer_half(rhs_pair, wpair, mh, outer_row=None):
                    """[128, CH] psum of rhs @ W for one output half; optional
                    K=1 outer-product accumulation (mean-row correction)."""
                    pm = pmm.tile([128, CH], f32, tag="mm", bufs=2, name="pm")
                    last = outer_row is None
                    for kh in range(2):
                        for o0, w_ in BANK_SUBS:
                            cs = slice(o0, o0 + w_)
                            nc.tensor.matmul(
                                pm[:, cs],
                                wpair[kh][:, mh * 128:(mh + 1) * 128],
                                rhs_pair[kh][:, cs],
                                start=(kh == 0), stop=(kh == 1) and last)
                    if outer_row is not None:
                        row, vec = outer_row
                        for o0, w_ in BANK_SUBS:
                            cs = slice(o0, o0 + w_)
                            nc.tensor.matmul(
                                pm[:, cs],
                                row[0:1, mh * 128:(mh + 1) * 128],
                                vec[0:1, cs],
                                start=False, stop=True)
                    return pm

                def square_pair(src_pair, smp_tag):
                    a2 = []
                    for hh in range(2):
                        t = p2.tile([128, CH], bf16, tag="a2", bufs=4,
                                    name=f"a2_{smp_tag}{hh}")
                        nc.vector.tensor_mul(t, src_pair[hh], src_pair[hh])
                        a2.append(t)
                    return a2

                def ln_stats(src_pair, a2, smp_tag):
                    """LN stats for src (pair of [128,CH] bf16, a2 = src**2
                    precomputed a stage earlier): returns (rstd [128,CH] f32
                    bc over partitions, mean_sb [1,CH] bf16). Normalization is
                    applied AFTER the next matmul: (src@Wg - mean*colsum(Wg))
                    * rstd."""
                    var_ = p2.tile([128, CH], f32, tag="var", bufs=2,
                                   name=f"var_{smp_tag}")
                    mean_sb = p2.tile([1, CH], bf16, tag="meanrow", bufs=4,
                                      name=f"mean_{smp_tag}")
                    for su in range(N_SUBW):
                        cs = slice(su * SUBW, (su + 1) * SUBW)
                        pmn = pst.tile([128, SUBW], f32, tag="stat", name="pmn")
                        for hh in range(2):
                            nc.tensor.matmul(pmn, onesC_b, src_pair[hh][:, cs],
                                             start=(hh == 0), stop=(hh == 1))
                        pms = pst.tile([128, SUBW], f32, tag="stat", name="pms")
                        for hh in range(2):
                            nc.tensor.matmul(pms, onesC_b, a2[hh][:, cs],
                                             start=(hh == 0), stop=(hh == 1))
                        nc.scalar.activation(out=mean_sb[0:1, cs], in_=pmn[0:1, :],
                                             func=AF.Copy)
                        m2s = p2.tile([128, SUBW], f32, tag="m2s", bufs=4,
                                      name=f"m2s_{smp_tag}")
                        nc.scalar.activation(out=m2s, in_=pmn, func=AF.Square)
                        nc.vector.tensor_sub(var_[:, cs], pms, m2s)
                    nc.scalar.activation(out=var_, in_=var_, func=AF.Sqrt,
                                         bias=eps_t)
                    rstd = p2.tile([128, CH], f32, tag="rstd", bufs=3,
                                   name=f"rstd_{smp_tag}")
                    nc.vector.reciprocal_approx_fast(out=rstd, in_=var_)
                    return rstd, mean_sb

                def back_stages(cc):
                    g0 = cc * CH_G
                    z = z_store.pop(cc)
                    # S1: att = z @ Wtp + px_unf
                    att = []
                    for mh in range(2):
                        pm = layer_half(z, wtp, mh)
                        ah = p2.tile([128, CH], bf16, tag="att", bufs=4, name="att")
                        nc.vector.scalar_tensor_tensor(
                            out=_view(ah, 0, [[L, CH_G], [1, L]]),
                            in0=_view(pm, 0, [[L, CH_G], [1, L]]),
                            scalar=0.0,
                            in1=unf(px_f[mh], g0, CH_G),
                            op0=ALU.add, op1=ALU.add)
                        att.append(ah)
                    yield
                    # S2: FFN LN stats
                    rstd1, mean1 = ln_stats(att, square_pair(att, f"f{cc}"), f"f{cc}")
                    yield
                    # S3/S4: p1 + rstd apply + gelu, one output half per stage
                    g1 = []
                    for mh in range(2):
                        pm = layer_half(att, w1g, mh, outer_row=(negg[0], mean1))
                        tg = p2.tile([128, CH], bf16, tag="tg", bufs=3, name="tg")
                        nc.vector.scalar_tensor_tensor(
                            out=tg, in0=pm, scalar=1.0, in1=rstd1,
                            op0=ALU.mult, op1=ALU.mult)
                        gh = p2.tile([128, CH], bf16, tag="g1", bufs=3, name="g1")
                        nc.scalar.activation(out=gh, in_=tg, func=AF.Gelu,
                                             bias=B1_t[:, mh:mh + 1])
                        g1.append(gh)
                        yield
                    # S5: p2 + residual
                    y = []
                    for mh in range(2):
                        pm = layer_half(g1, w2b, mh)
                        ysc = p2.tile([128, CH], bf16, tag="ysc", bufs=2, name="ysc")
                        nc.scalar.activation(out=ysc, in_=pm, func=AF.Identity,
                                             bias=bias["b2"][:, mh:mh + 1])
                        yh = p2.tile([128, CH], bf16, tag="ytag", bufs=3, name="y")
                        nc.gpsimd.tensor_add(yh, ysc, att[mh])
                        y.append(yh)
                    yield
                    # S6: temporal LN stats
                    rstd2, mean2 = ln_stats(y, square_pair(y, f"t{cc}"), f"t{cc}")
                    yield
                    # S7/S8: p3 + rstd apply + gelu into w-major layout
                    h_act = []
                    for mh in range(2):
                        pm = layer_half(y, c1g, mh, outer_row=(negg[1], mean2))
                        tg2 = p2.tile([128, CH], bf16, tag="tg", bufs=3, name="tg2")
                        nc.vector.scalar_tensor_tensor(
                            out=tg2, in0=pm, scalar=1.0, in1=rstd2,
                            op0=ALU.mult, op1=ALU.mult)
                        hh_ = p2.tile([128, CH], bf16, tag="hact", bufs=2, name="h_act")
                        dst = _view(hh_, 0, [[V, CH_G], [CH_G * V, W], [1, V]])
                        nc.scalar.activation(out=dst, in_=tg2, func=AF.Gelu,
                                             bias=Bc1_t[:, mh:mh + 1])
                        h_act.append(hh_)
                        yield
                    # S9: c2 contraction (w, i) -> out [O, CH_G*V]
                    for mh in range(2):
                        po = pst.tile([128, CH_G * V], f32, tag="stat", name="po")
                        first = True
                        for w in range(W):
                            for kh in range(2):
                                rhs = h_act[kh][:, w * CH_G * V:(w + 1) * CH_G * V]
                                nc.tensor.matmul(po, c2b[w][kh][:, mh * 128:(mh + 1) * 128],
                                                 rhs, start=first,
                                                 stop=(w == W - 1 and kh == 1))
                                first = False
                        os_ = p2.tile([128, CH_G * V], f32, tag="os", bufs=2, name="os_")
                        nc.scalar.activation(out=os_, in_=po, func=AF.Identity,
                                             bias=bias["c2_b"][:, mh:mh + 1])
                        nc.sync.dma_start(
                            out=out_d[mh * 128:(mh + 1) * 128, g0:g0 + CH_G, :],
                            in_=os_)

                NSTAGE = 9
                START = [0, 2, 4, 6, 8, 10, 12, 14]
                gens = [back_stages(cc) for cc in range(N_CH)]
                front_sched = {}
                for cc in range(2, N_CH):
                    r1a = max(0, START[cc] - 6)
                    front_sched.setdefault(r1a, []).append(lambda c=cc: front_1a(c))
                    front_sched.setdefault(max(r1a + 1, START[cc] - 5), []).append(lambda c=cc: front_1b(c))
                    front_sched.setdefault(max(r1a + 2, START[cc] - 3), []).append(lambda c=cc: front_2(c))
                for r in range(START[-1] + NSTAGE):
                    # back stages first: their psum-consumer ops land early in
                    # the engine FIFOs; latency-tolerant front chains go last
                    for cc in range(N_CH):
                        if 0 <= r - START[cc] < NSTAGE:
                            next(gens[cc], None)
                    for fn in front_sched.get(r, []):
                        fn()

            fps_cm.__exit__(None, None, None)
            frp_cm.__exit__(None, None, None)
    return nc


_CACHE = {}


def _get_compiled():
    if "nc" not in _CACHE:
        nc = bacc.Bacc("TRN2", target_bir_lowering=False, debug=False)
        build(nc)
        nc.compile()
        _CACHE["nc"] = nc
    return _CACHE["nc"]


def kernel(**inputs):
    nc = _get_compiled()
    x = np.asarray(inputs["x"], dtype=np.float32)
    n = x.shape[0]
    names = ["Wq", "Wk", "Wv", "Wt", "Wp", "W1", "W2", "c1_w", "Wqa", "Wka",
             "c2_w", "ln1_g", "ln1_b", "bq", "bk", "bv", "bt", "bp", "ffn_g",
             "ffn_b", "b1", "b2", "tn_g", "tn_b", "c1_b", "c2_b", "bqa", "bka"]
    shared = {nm: np.asarray(inputs[nm], dtype=np.float32) for nm in names}
    in_maps = [{"x": x[i], **shared} for i in range(n)]
    res = bass_utils.run_bass_kernel_spmd(nc, in_maps, core_ids=list(range(n)))
    return np.stack([res.results[i]["out"] for i in range(n)], axis=0)


if __name__ == "__main__":
    nc = bacc.Bacc("TRN2", target_bir_lowering=False, debug=False)
    build(nc)
    nc.compile()
    print("build+compile OK")



# revision 52
# speedup vs baseline: 1.0022x; 1.0022x over previous
"""Trainium2 Bass kernel for nn_Block_47193100648803.

Contract: kernel(**inputs) takes FULL unsharded inputs (numpy), returns the
FULL (N, O, T, V) output. Internally shards data-parallel over N across the
8 NeuronCores (one batch element per core, weights replicated).

Layout: channels on SBUF partitions (C=256 -> 2 half-tiles of 128), tokens on
the free axis; the temporal-window unfold is expressed with overlapping
strided access patterns (no data movement). All dense matmuls run bf16
(full PE rate); LN statistics come from PE ones-matmuls, rstd = Sqrt +
reciprocal_approx_fast, and normalization is applied AFTER the following
matmul ((x@Wg - mean*colsum(Wg))*rstd), so stats and main matmuls are
independent PE work. Wt@Wp is pre-fused; the pooled-attention "fronts"
(softmax via DRAM-bounce transpose DMAs, pooled q/k) are split into
latency-tolerant stages; ka logits use a Wka x pq outer-product so the k*pq
product is never materialized.

Schedule: phase 1 produces q/k/v/px per frame-tile with the first chunks'
fronts interleaved; phase 2 drives each chunk's 9-stage back pipeline
(att -> FFN-LN -> W1+gelu -> W2+residual -> temporal-LN -> c1+gelu -> c2)
as generators skewed 2 rounds apart, backs emitted before fronts in each
round so psum-consumer ops sit early in the engine FIFOs. Elementwise work
is spread across vector/gpsimd/scalar to keep the PE's in-order stream fed
(the HAM clock gate halves the PE clock after ~3.4us idle windows).
"""

import os
import sys

import numpy as np

for _p in ("/opt/trn_rl_repo", "/root/.axon_site/_ro/trn_rl_repo"):
    if os.path.isdir(_p) and _p not in sys.path:
        sys.path.append(_p)

import concourse.bass as bass
import concourse.tile as tile
from concourse import bacc, bass_utils, mybir
from concourse.masks import make_identity

f32 = mybir.dt.float32
f32r = mybir.dt.float32r
bf16 = mybir.dt.bfloat16
AF = mybir.ActivationFunctionType
ALU = mybir.AluOpType
AX = mybir.AxisListType

# ---- problem constants (hardcoded per spec) ----
N_CORES = 8
C, T, V = 256, 128, 25
H = 8
W = 3
O = 256
L = W * V                 # 75
FT = T + 2                # 130 padded frames
F = FT * V                # 3250 real frame columns (zero pads at both ends)
F_PAD = 3328              # allocated frame columns (8 * 416, fp32r-even subs)
G = T                     # 128 groups per core
GL = G * L                # 9600 group-stage columns
SCALE = 1.0 / (32.0 ** 0.5)
EPS = 1e-5

FSUB = 416                # phase-1 matmul column tile (even, 256..512)
N_FSUB = F_PAD // FSUB    # 8
CH_G = 16                 # groups per chunk in phase 2
N_CH = G // CH_G          # 8
CH = CH_G * L             # 1200
SUB_G = 4                 # groups per matmul sub-tile
SUB = SUB_G * L           # 300
N_SUB = CH_G // SUB_G     # 4
SUBW = 400                # wide matmul sub (fp32r-even, 256..512)
N_SUBW = CH // SUBW       # 3
# bank-aligned dst slices for chunk-wide [128, CH] psum accumulation
BANK_SUBS = [(0, 512), (512, 512), (1024, 176)]


def _r(ap):
    return ap.bitcast(f32r)


def _view(t, offset, dims):
    """AP view on tile t: partition dim kept, free dims replaced."""
    return bass.AP(tensor=t.tensor, offset=t.offset + offset, ap=[t.ap[0]] + dims)


def unf(t, g0, gc):
    """Overlapping window view [128, gc, W, V] on a [128, F] frame tile."""
    return _view(t, g0 * V, [[V, gc], [V, W], [1, V]])


def seg(t, g0, gc):
    """[128, gc, L] view on a [128, GL] or chunk tile starting at group g0
    (g0 relative to tile origin)."""
    return _view(t, g0 * L, [[L, gc], [1, L]])


def bc_g(t, g0, gc):
    """Broadcast per-(c,g) [128, G] tile over L -> [128, gc, L] (step-0)."""
    return _view(t, g0, [[1, gc], [0, L]])


def build(nc):
    x_d = nc.dram_tensor("x", [C, T, V], f32, kind="ExternalInput").ap()
    wd = {}
    for nm in ["Wq", "Wk", "Wv", "Wt", "Wp", "W1", "W2", "c1_w"]:
        wd[nm] = nc.dram_tensor(nm, [C, C], f32, kind="ExternalInput").ap()
    wd["Wqa"] = nc.dram_tensor("Wqa", [C, H], f32, kind="ExternalInput").ap()
    wd["Wka"] = nc.dram_tensor("Wka", [C, H], f32, kind="ExternalInput").ap()
    wd["c2_w"] = nc.dram_tensor("c2_w", [W, C, O], f32, kind="ExternalInput").ap()
    bnames = ["ln1_g", "ln1_b", "bq", "bk", "bv", "bt", "bp", "ffn_g", "ffn_b",
              "b1", "b2", "tn_g", "tn_b", "c1_b", "c2_b"]
    for nm in bnames:
        wd[nm] = nc.dram_tensor(nm, [C], f32, kind="ExternalInput").ap()
    wd["bqa"] = nc.dram_tensor("bqa", [H], f32, kind="ExternalInput").ap()
    wd["bka"] = nc.dram_tensor("bka", [H], f32, kind="ExternalInput").ap()
    out_d = nc.dram_tensor("out", [O, T, V], f32, kind="ExternalOutput").ap()

    dbg = os.environ.get("BASS_DEBUG_SCRATCH") == "1"
    skind = "ExternalOutput" if dbg else "Internal"
    qa_d = nc.dram_tensor("qa_scr", [H, F_PAD], f32, kind=skind).ap()
    qw_d = nc.dram_tensor("qw_scr", [H, GL], bf16, kind=skind).ap()
    ka_d = nc.dram_tensor("ka_scr", [H, GL], f32, kind=skind).ap()
    kw_d = nc.dram_tensor("kw_scr", [H, GL], bf16, kind=skind).ap()
    row_d = nc.dram_tensor("row_scr", [3, C], f32).ap()

    with tile.TileContext(nc) as tc:
        with (
            tc.tile_pool(name="consts", bufs=1) as cp,
            tc.tile_pool(name="data", bufs=1) as dp,
        ):
            # front pools span both phases; entered before p1x for LIFO exit
            frp_cm = tc.tile_pool(name="front_sb", bufs=1)
            frp = frp_cm.__enter__()
            fps_cm = tc.tile_pool(name="front_ps", bufs=2, space="PSUM")
            fps = fps_cm.__enter__()

            # ---------- input load first (weights go on other DMA queues) ----
            p1x_cm = tc.tile_pool(name="p1_x", bufs=1)
            p1x = p1x_cm.__enter__()
            x_f = [p1x.tile([128, F_PAD], f32, tag=f"x_f{hh}", name=f"x_f{hh}")
                   for hh in range(2)]
            for hh in range(2):
                for qt in range(4):
                    ts0, ts1 = qt * (T // 4), (qt + 1) * (T // 4)
                    nc.gpsimd.dma_start(
                        out=_r(x_f[hh][:, V + ts0 * V:V + ts1 * V]),
                        in_=_r(x_d[hh * 128:(hh + 1) * 128, ts0:ts1, :]))
                nc.vector.memset(x_f[hh][:, 0:V], 0.0)
                nc.vector.memset(x_f[hh][:, F - V:F_PAD], 0.0)

            # ---------- weights / constants ----------
            # f32 masters live in a staging pool freed after setup; only the
            # bf16 working copies persist.
            wsp_cm = tc.tile_pool(name="wstage", bufs=1)
            wsp = wsp_cm.__enter__()
            wt = {}
            for nm in ["Wt", "Wp", "Wq", "Wk", "Wv", "W1", "W2", "c1_w"]:
                wt[nm] = [wsp.tile([128, C], f32, tag=f"w_{nm}{kh}", name=f"w_{nm}{kh}")
                          for kh in range(2)]
                for kh in range(2):
                    nc.sync.dma_start(out=_r(wt[nm][kh]),
                                      in_=_r(wd[nm][kh * 128:(kh + 1) * 128, :]))
            for nm in ["Wqa", "Wka"]:
                wt[nm] = [wsp.tile([128, H], f32, tag=f"w_{nm}{kh}", name=f"w_{nm}{kh}")
                          for kh in range(2)]
                for kh in range(2):
                    nc.sync.dma_start(out=_r(wt[nm][kh]),
                                      in_=_r(wd[nm][kh * 128:(kh + 1) * 128, :]))
            c2t = []
            for w in range(W):
                c2t.append([wsp.tile([128, O], f32, tag=f"w_c2_{w}{kh}", name=f"w_c2_{w}{kh}")
                            for kh in range(2)])
                for kh in range(2):
                    nc.sync.dma_start(out=_r(c2t[w][kh]),
                                      in_=_r(wd["c2_w"][w, kh * 128:(kh + 1) * 128, :]))

            def load_bias_col(nm):
                t = cp.tile([128, 2], f32, tag=f"b_{nm}", name=f"b_{nm}")
                src = bass.AP(tensor=wd[nm].tensor, offset=wd[nm].offset,
                              ap=[[1, 128], [128, 2]])
                nc.gpsimd.dma_start(out=t, in_=src)
                return t

            bias = {nm: load_bias_col(nm) for nm in bnames}
            for nm in ["bqa", "bka"]:
                t = cp.tile([H, 1], f32, tag=f"b_{nm}", name=f"b_{nm}")
                nc.sync.dma_start(out=t, in_=wd[nm])
                bias[nm] = t

            eps_t = cp.tile([128, 1], f32, tag="eps", name="eps_t")
            nc.vector.memset(eps_t, EPS)

            def fill_r(t, value):
                # constant fill with an f32r-typed output (plain Memset cannot
                # emit f32r): Copy(in*0 + value) ignores the uninitialized in_
                nc.scalar.activation(out=_r(t), in_=_r(t), func=AF.Copy,
                                     bias=float(value), scale=0.0)

            onesC = cp.tile([128, 128], f32, tag="onesC", name="onesC")
            fill_r(onesC, 1.0 / C)
            onesC_b = cp.tile([128, 128], bf16, tag="onesC_b", name="onesC_b")
            nc.scalar.activation(out=onesC_b, in_=onesC, func=AF.Copy)
            w2b = [cp.tile([128, C], bf16, tag=f"w2b{kh}", name=f"w2b{kh}") for kh in range(2)]
            wkab = [cp.tile([128, H], bf16, tag=f"wkab{kh}", name=f"wkab{kh}") for kh in range(2)]
            negones = cp.tile([128, 1], f32, tag="negones", name="negones")
            fill_r(negones, -1.0)
            negones_b = cp.tile([128, 1], bf16, tag="negones_b", name="negones_b")
            nc.scalar.activation(out=negones_b, in_=negones, func=AF.Copy)

            # folded weights: W1g = ffn_g*W1, c1g = tn_g*c1_w
            w1g = [cp.tile([128, C], bf16, tag=f"w1g{kh}", name=f"w1g{kh}") for kh in range(2)]
            c1g = [cp.tile([128, C], bf16, tag=f"c1g{kh}", name=f"c1g{kh}") for kh in range(2)]
            for kh in range(2):
                nc.vector.tensor_scalar_mul(w1g[kh], wt["W1"][kh], bias["ffn_g"][:, kh:kh + 1])
                nc.vector.tensor_scalar_mul(c1g[kh], wt["c1_w"][kh], bias["tn_g"][:, kh:kh + 1])
                nc.vector.tensor_copy(w2b[kh], wt["W2"][kh])
                nc.vector.tensor_copy(wkab[kh], wt["Wka"][kh])

            # bf16 copies of the phase-1 matmul weights (full-rate PE)
            wb = {}
            for nm in ["Wq", "Wk", "Wv", "Wp"]:
                wb[nm] = [cp.tile([128, C], bf16, tag=f"wb_{nm}{kh}", name=f"wb_{nm}{kh}")
                          for kh in range(2)]
                for kh in range(2):
                    nc.vector.tensor_copy(wb[nm][kh], wt[nm][kh])
            wqab = [cp.tile([128, H], bf16, tag=f"wqab{kh}", name=f"wqab{kh}") for kh in range(2)]
            for kh in range(2):
                nc.vector.tensor_copy(wqab[kh], wt["Wqa"][kh])
            # Wka replicated over the chunk's groups, g-major: [c, g, h]
            wkarep_f = [cp.tile([128, CH_G * H], f32, tag=f"wkarep_f{kh}",
                                name=f"wkarep_f{kh}") for kh in range(2)]
            wkarep = [cp.tile([128, CH_G * H], bf16, tag=f"wkarep{kh}",
                              name=f"wkarep{kh}") for kh in range(2)]
            for kh in range(2):
                srcr = bass.AP(tensor=wd["Wka"].tensor,
                               offset=wd["Wka"].offset + kh * 128 * H,
                               ap=[[H, 128], [0, CH_G], [1, H]])
                nc.sync.dma_start(out=_view(wkarep_f[kh], 0, [[H, CH_G], [1, H]]),
                                  in_=srcr)
                nc.vector.tensor_copy(wkarep[kh], wkarep_f[kh])
            c2b = []
            for w in range(W):
                c2b.append([cp.tile([128, O], bf16, tag=f"wb_c2_{w}{kh}", name=f"wb_c2_{w}{kh}")
                            for kh in range(2)])
                for kh in range(2):
                    nc.vector.tensor_copy(c2b[w][kh], c2t[w][kh])

            wtp = [cp.tile([128, C], bf16, tag=f"wtp{kh}", name=f"wtp{kh}") for kh in range(2)]
            negg = [cp.tile([1, C], bf16, tag=f"negg{i}", name=f"negg{i}")
                    for i in range(2)]  # [-G1], [-Gc1]

            # ---------- setup-scoped: Wtp = Wt@Wp, bias rows ----------
            with (
                tc.tile_pool(name="setup_sb", bufs=1) as sp,
                tc.tile_pool(name="setup_ps", bufs=2, space="PSUM") as spp,
            ):
                wtw = wt["Wt"]
                ident = sp.tile([128, 128], f32, tag="ident", name="ident")
                make_identity(nc, ident)

                for kh in range(2):
                    pacc = spp.tile([128, C], f32, tag="wtp_acc", name="pacc")
                    for mh in range(2):
                        ptr = spp.tile([128, 128], f32, tag="tr", name="ptr")
                        nc.tensor.transpose(ptr, wtw[kh][:, mh * 128:(mh + 1) * 128], ident)
                        a_t = sp.tile([128, 128], f32, tag="a_t", name="a_t")
                        nc.scalar.activation(out=_r(a_t), in_=ptr, func=AF.Copy)
                        nc.tensor.matmul(pacc, _r(a_t), _r(wt["Wp"][mh]),
                                         start=(mh == 0), stop=(mh == 1))
                    nc.scalar.activation(out=wtp[kh], in_=pacc, func=AF.Copy)

                def colvec(nm, kh):
                    t = sp.tile([128, 1], f32, tag=f"cv_{nm}{kh}", name=f"cv_{nm}{kh}")
                    src = bass.AP(tensor=wd[nm].tensor, offset=wd[nm].offset + kh * 128,
                                  ap=[[1, 128], [128, 1]])
                    nc.sync.dma_start(out=_r(t), in_=_r(src))
                    return t

                def rowvec(nm):
                    t = sp.tile([1, C], f32, tag=f"rv_{nm}", name=f"rv_{nm}")
                    nc.sync.dma_start(out=t, in_=wd[nm])
                    return t

                for i, (bnm, wmat, addnm) in enumerate([
                    ("bt", wt["Wp"], "bp"),
                    ("ffn_b", wt["W1"], "b1"),
                    ("tn_b", wt["c1_w"], "c1_b"),
                ]):
                    pr = spp.tile([1, C], f32, tag="rowacc", name="pr")
                    for kh in range(2):
                        nc.tensor.matmul(pr, _r(colvec(bnm, kh)), _r(wmat[kh]),
                                         start=(kh == 0), stop=(kh == 1))
                    row_i = sp.tile([1, C], f32, tag=f"row_i{i}", name=f"row_i{i}")
                    nc.vector.tensor_add(row_i, pr, rowvec(addnm))
                    nc.sync.dma_start(out=row_d[i:i + 1, :], in_=row_i)

                for i, wmat in enumerate([w1g, c1g]):
                    pg = spp.tile([1, C], f32, tag="rowacc", name="pg")
                    for kh in range(2):
                        nc.tensor.matmul(pg, negones_b, wmat[kh],
                                         start=(kh == 0), stop=(kh == 1))
                    nc.scalar.activation(out=negg[i], in_=pg, func=AF.Copy)

            wsp_cm.__exit__(None, None, None)

            # bounce bias rows back into per-partition [128, 2] layout
            btp_t = cp.tile([128, 2], f32, tag="btp", name="btp_t")
            B1_t = cp.tile([128, 2], f32, tag="B1", name="B1_t")
            Bc1_t = cp.tile([128, 2], f32, tag="Bc1", name="Bc1_t")
            for i, t in enumerate([btp_t, B1_t, Bc1_t]):
                src = bass.AP(tensor=row_d.tensor, offset=row_d.offset + i * C,
                              ap=[[1, 128], [128, 1]])
                nc.sync.dma_start(out=t[:, 0:1], in_=src)
                src2 = bass.AP(tensor=row_d.tensor, offset=row_d.offset + i * C + 128,
                               ap=[[1, 128], [128, 1]])
                nc.sync.dma_start(out=t[:, 1:2], in_=src2)

            # ---------- persistent activations ----------
            px_f = [dp.tile([128, F_PAD], bf16, tag=f"px_f{hh}", name=f"px_f{hh}") for hh in range(2)]
            q_f = [dp.tile([128, F_PAD], bf16, tag=f"q_f{hh}", name=f"q_f{hh}") for hh in range(2)]
            k_f = [dp.tile([128, F_PAD], bf16, tag=f"k_f{hh}", name=f"k_f{hh}") for hh in range(2)]
            v_f = [dp.tile([128, F_PAD], bf16, tag=f"v_f{hh}", name=f"v_f{hh}") for hh in range(2)]
            pq_b = [dp.tile([128, G], bf16, tag=f"pqb{hh}", name=f"pqb{hh}") for hh in range(2)]
            pk_b = [dp.tile([128, G], bf16, tag=f"pkb{hh}", name=f"pkb{hh}") for hh in range(2)]

            # ---------- chunk fronts (outer scope: used in both phases) ------

            def softmax_chunk(src_gather_ap, dst_dram, g0, tagp):
                """Per-chunk softmax in [128 = 16 groups x 8 heads, L] layout;
                writes normalized weights to dst_dram[h, cols]. Logits are
                tiny (|x|*scale << 1) so no max-subtraction is needed; the
                elementwise ops run on gpsimd to keep latency off the busy
                vector queue."""
                ag = frp.tile([128, L], f32, tag="sm_ag", bufs=4,
                              name=f"ag_{tagp}")
                nc.gpsimd.dma_start(out=ag, in_=src_gather_ap)
                e = frp.tile([128, L], f32, tag="sm_e", bufs=4,
                             name=f"e_{tagp}")
                nc.scalar.activation(out=e, in_=ag, func=AF.Exp, scale=SCALE)
                sm = frp.tile([128, 1], f32, tag="sm_s", bufs=4,
                              name=f"sm_{tagp}")
                nc.vector.reduce_sum(sm, e, axis=AX.X)
                rs = frp.tile([128, 1], f32, tag="sm_rs", bufs=4,
                              name=f"rs_{tagp}")
                nc.vector.reciprocal(rs, sm)
                wgn = frp.tile([128, L], bf16, tag="sm_w", bufs=4,
                               name=f"wgn_{tagp}")
                nc.gpsimd.tensor_scalar_mul(wgn, e, rs[:, 0:1])
                dst = bass.AP(tensor=dst_dram.tensor,
                              offset=dst_dram.offset + g0 * L,
                              ap=[[L, CH_G], [GL, H], [1, L]])
                nc.gpsimd.dma_start(out=dst, in_=wgn)

            def head_bcast(src_dram, g0, hh, tagp):
                """[128, CH] tile with partition c reading
                src_dram[c // 32 (+4*hh), chunk cols] via broadcast DMA."""
                t = frp.tile([128, CH], bf16, tag="bc", bufs=2,
                             name=f"bc_{tagp}")
                src = bass.AP(
                    tensor=src_dram.tensor,
                    offset=src_dram.offset + (hh * 4) * GL + g0 * L,
                    ap=[[GL, 4], [0, 32], [1, CH]])
                nc.sync.dma_start(out=t, in_=src)
                return t

            kp_store = {}
            z_store = {}

            def front_1a(cc):
                """qw softmax -> pooled query -> kp = k*pq (no PE work)."""
                g0 = cc * CH_G
                qa_gather = bass.AP(
                    tensor=qa_d.tensor, offset=qa_d.offset + g0 * V,
                    ap=[[V, CH_G], [F_PAD, H], [V, W], [1, V]])
                softmax_chunk(qa_gather, qw_d, g0, f"q{cc}")
                kp = []
                for hh in range(2):
                    qb = head_bcast(qw_d, g0, hh, f"q{cc}{hh}")
                    prod = frp.tile([128, CH], bf16, tag="prod", bufs=2,
                                    name="prod")
                    nc.gpsimd.tensor_mul(_view(prod, 0, [[L, CH_G], [1, L]]),
                                         unf(q_f[hh], g0, CH_G),
                                         _view(qb, 0, [[L, CH_G], [1, L]]))
                    with nc.allow_low_precision(reason="DVE reduce accumulates fp32; bf16 rounding only at output"):
                        nc.vector.reduce_sum(pq_b[hh][:, g0:g0 + CH_G],
                                             _view(prod, 0, [[L, CH_G], [1, L]]),
                                             axis=AX.X)
                    # kwc[c, g, h] = Wka[c,h] * pq[c,g]: tiny outer product;
                    # ka then contracts k-frames directly (no k*pq materialize)
                    kwc = frp.tile([128, CH_G * H], bf16, tag="kwc", bufs=2,
                                   name="kwc")
                    nc.vector.scalar_tensor_tensor(
                        out=kwc, in0=wkarep[hh], scalar=1.0,
                        in1=_view(pq_b[hh], g0, [[1, CH_G], [0, H]]),
                        op0=ALU.mult, op1=ALU.mult)
                    kp.append(kwc)
                kp_store[cc] = kp

            def front_1b(cc):
                """ka = kp @ Wka -> DRAM (PE work, one round after F1a)."""
                g0 = cc * CH_G
                col0 = g0 * L
                kwc = kp_store.pop(cc)
                ka_c = frp.tile([H, CH], f32, tag="ka_c", bufs=2, name="ka_c")
                for su in range(4):
                    pka = fps.tile([H, SUB], f32, tag="stat", name="pka")
                    for gi in range(SUB_G):
                        g = su * SUB_G + gi
                        ls = slice(gi * L, (gi + 1) * L)
                        for kh in range(2):
                            nc.tensor.matmul(
                                pka[:, ls],
                                _view(kwc[kh], g * H, [[1, H]]),
                                k_f[kh][:, (g0 + g) * V:(g0 + g) * V + L],
                                start=(kh == 0), stop=(kh == 1))
                    nc.scalar.activation(out=ka_c[:, su * SUB:(su + 1) * SUB],
                                         in_=pka, func=AF.Identity, bias=bias["bka"])
                nc.gpsimd.dma_start(out=ka_d[:, col0:col0 + CH], in_=ka_c)

            def front_2(cc):
                """kw softmax -> pooled key -> z = v * pk."""
                g0 = cc * CH_G
                col0 = g0 * L
                ka_gather = bass.AP(
                    tensor=ka_d.tensor, offset=ka_d.offset + col0,
                    ap=[[L, CH_G], [GL, H], [1, L]])
                softmax_chunk(ka_gather, kw_d, g0, f"k{cc}")
                z = []
                for hh in range(2):
                    kb = head_bcast(kw_d, g0, hh, f"k{cc}{hh}")
                    prod = frp.tile([128, CH], bf16, tag="prod", bufs=2,
                                    name="prod2")
                    nc.gpsimd.tensor_mul(_view(prod, 0, [[L, CH_G], [1, L]]),
                                         unf(k_f[hh], g0, CH_G),
                                         _view(kb, 0, [[L, CH_G], [1, L]]))
                    with nc.allow_low_precision(reason="DVE reduce accumulates fp32; bf16 rounding only at output"):
                        nc.vector.reduce_sum(pk_b[hh][:, g0:g0 + CH_G],
                                             _view(prod, 0, [[L, CH_G], [1, L]]),
                                             axis=AX.X)
                    zh = frp.tile([128, CH], bf16, tag="z", bufs=6,
                                  name=f"z{cc}_{hh}")
                    nc.vector.scalar_tensor_tensor(
                        out=_view(zh, 0, [[L, CH_G], [1, L]]),
                        in0=unf(v_f[hh], g0, CH_G), scalar=1.0,
                        in1=bc_g(pk_b[hh], g0, CH_G),
                        op0=ALU.mult, op1=ALU.mult)
                    z.append(zh)
                z_store[cc] = z

            # fronts for chunks 0/1 run in the phase-1 tail; later fronts are
            # scheduled inside the phase-2 driver on PE-quiet rounds
            P1_FRONT = {2: [lambda: front_1a(0)], 3: [lambda: front_1b(0)],
                        4: [lambda: front_2(0)], 5: [lambda: front_1a(1)],
                        6: [lambda: front_1b(1)], 7: [lambda: front_2(1)]}

            # ---------- phase 1: per-frame pipeline + chunk fronts ----------
            with (
                tc.tile_pool(name="p1_sb", bufs=2) as p1,
                tc.tile_pool(name="p1_ps", bufs=1, space="PSUM") as pp1,
                tc.tile_pool(name="p1_mm", bufs=3, space="PSUM") as pp1m,
            ):
                for s in range(N_FSUB):
                    sl = slice(s * FSUB, (s + 1) * FSUB)
                    x2 = [p1.tile([128, FSUB], bf16, tag=f"x2_{hh}", name=f"x2_{hh}")
                          for hh in range(2)]
                    for hh in range(2):
                        nc.vector.scalar_tensor_tensor(
                            out=x2[hh], in0=x_f[hh][:, sl], scalar=1.0,
                            in1=x_f[hh][:, sl], op0=ALU.mult, op1=ALU.mult)
                    pmean = pp1.tile([128, FSUB], f32, tag="pmean", name="pmean")
                    pmsq = pp1.tile([128, FSUB], f32, tag="pmsq", name="pmsq")
                    for hh in range(2):
                        nc.tensor.matmul(pmean, _r(onesC), _r(x_f[hh][:, sl]),
                                         start=(hh == 0), stop=(hh == 1))
                    for hh in range(2):
                        nc.tensor.matmul(pmsq, onesC_b, x2[hh],
                                         start=(hh == 0), stop=(hh == 1))
                    m2 = p1.tile([128, FSUB], f32, tag="m2", name="m2")
                    nc.scalar.activation(out=m2, in_=pmean, func=AF.Square)
                    var = p1.tile([128, FSUB], f32, tag="var", name="var")
                    nc.vector.tensor_sub(var, pmsq, m2)
                    srt = p1.tile([128, FSUB], f32, tag="sd", name="srt")
                    nc.scalar.activation(out=srt, in_=var, func=AF.Sqrt, bias=eps_t)
                    rstd = p1.tile([128, FSUB], f32, tag="rstd", name="rstd")
                    nc.vector.reciprocal_approx_fast(out=rstd, in_=srt)
                    nx = []
                    for hh in range(2):
                        xc = p1.tile([128, FSUB], f32, tag=f"xc{hh}", name=f"xc{hh}")
                        nc.vector.tensor_sub(xc, x_f[hh][:, sl], pmean)
                        xg = p1.tile([128, FSUB], f32, tag=f"xg{hh}", name=f"xg{hh}")
                        nc.vector.scalar_tensor_tensor(
                            out=xg, in0=xc, scalar=bias["ln1_g"][:, hh:hh + 1],
                            in1=rstd, op0=ALU.mult, op1=ALU.mult)
                        nxh = p1.tile([128, FSUB], bf16, tag=f"nx{hh}", name=f"nx{hh}")
                        nc.vector.tensor_scalar_add(nxh, xg, bias["ln1_b"][:, hh:hh + 1])
                        nx.append(nxh)
                    for mh in range(2):
                        pq_ = pp1m.tile([128, FSUB], f32, tag="mm", name="pq_")
                        for kh in range(2):
                            nc.tensor.matmul(pq_, wb["Wq"][kh][:, mh * 128:(mh + 1) * 128],
                                             nx[kh], start=(kh == 0), stop=(kh == 1))
                        nc.scalar.activation(out=q_f[mh][:, sl], in_=pq_, func=AF.Identity,
                                             bias=bias["bq"][:, mh:mh + 1])
                    for nm, bnm, dst in [("Wk", "bk", k_f), ("Wv", "bv", v_f)]:
                        for mh in range(2):
                            pm_ = pp1m.tile([128, FSUB], f32, tag="mm", name="pm_")
                            for kh in range(2):
                                nc.tensor.matmul(pm_,
                                                 wb[nm][kh][:, mh * 128:(mh + 1) * 128],
                                                 nx[kh], start=(kh == 0), stop=(kh == 1))
                            nc.scalar.activation(out=dst[mh][:, sl], in_=pm_,
                                                 func=AF.Identity,
                                                 bias=bias[bnm][:, mh:mh + 1])
                    pqa = pp1.tile([H, FSUB], f32, tag="pqa", name="pqa")
                    for kh in range(2):
                        nc.tensor.matmul(pqa, wqab[kh], nx[kh],
                                         start=(kh == 0), stop=(kh == 1))
                    qa_s = p1.tile([H, FSUB], f32, tag="qa_s", name="qa_s")
                    nc.scalar.activation(out=qa_s, in_=pqa, func=AF.Identity,
                                         bias=bias["bqa"])
                    nc.sync.dma_start(out=qa_d[:, sl], in_=qa_s)
                    # px = q@Wp + btp + x   (pre-added residual path for attn)
                    for mh in range(2):
                        pp_ = pp1m.tile([128, FSUB], f32, tag="mm", name="pp_")
                        for kh in range(2):
                            nc.tensor.matmul(pp_, wb["Wp"][kh][:, mh * 128:(mh + 1) * 128],
                                             q_f[kh][:, sl], start=(kh == 0), stop=(kh == 1))
                        nc.vector.scalar_tensor_tensor(
                            out=px_f[mh][:, sl], in0=pp_, scalar=btp_t[:, mh:mh + 1],
                            in1=x_f[mh][:, sl], op0=ALU.add, op1=ALU.add)
                    for fn in P1_FRONT.get(s, []):
                        fn()
            p1x_cm.__exit__(None, None, None)

            # ---------- phase 2: skewed back pipeline ----------
            # Each chunk's back half is a 9-stage generator; chunks are driven
            # with a 4-stage skew so (stats, stats) and (gelu, gelu) stages of
            # neighboring chunks are adjacent in the scalar stream (activation
            # table sets batch: sqrt-set then gelu-set, 2 loads per chunk) and
            # the PE always has an independent matmul stream to fill stalls.
            with (
                tc.tile_pool(name="p2_sb", bufs=1) as p2,
                tc.tile_pool(name="p2_ps", bufs=2, space="PSUM") as pmm,
            ):
                pst = fps
                def layer_half(rhs_pair, wpair, mh, outer_row=None):
                    """[128, CH] psum of rhs @ W for one output half; optional
                    K=1 outer-product accumulation (mean-row correction)."""
                    pm = pmm.tile([128, CH], f32, tag="mm", bufs=2, name="pm")
                    last = outer_row is None
                    for kh in range(2):
                        for o0, w_ in BANK_SUBS:
                            cs = slice(o0, o0 + w_)
                            nc.tensor.matmul(
                                pm[:, cs],
                                wpair[kh][:, mh * 128:(mh + 1) * 128],
                                rhs_pair[kh][:, cs],
                                start=(kh == 0), stop=(kh == 1) and last)
                    if outer_row is not None:
                        row, vec = outer_row
                        for o0, w_ in BANK_SUBS:
                            cs = slice(o0, o0 + w_)
                            nc.tensor.matmul(
                                pm[:, cs],
                                row[0:1, mh * 128:(mh + 1) * 128],
                                vec[0:1, cs],
                                start=False, stop=True)
                    return pm

                def square_pair(src_pair, smp_tag):
                    a2 = []
                    for hh in range(2):
                        t = p2.tile([128, CH], bf16, tag="a2", bufs=4,
                                    name=f"a2_{smp_tag}{hh}")
                        nc.vector.tensor_mul(t, src_pair[hh], src_pair[hh])
                        a2.append(t)
                    return a2

                def ln_stats(src_pair, a2, smp_tag):
                    """LN stats for src (pair of [128,CH] bf16, a2 = src**2
                    precomputed a stage earlier): returns (rstd [128,CH] f32
                    bc over partitions, mean_sb [1,CH] bf16). Normalization is
                    applied AFTER the next matmul: (src@Wg - mean*colsum(Wg))
                    * rstd."""
                    var_ = p2.tile([128, CH], f32, tag="var", bufs=2,
                                   name=f"var_{smp_tag}")
                    mean_sb = p2.tile([1, CH], bf16, tag="meanrow", bufs=4,
                                      name=f"mean_{smp_tag}")
                    for su in range(N_SUBW):
                        cs = slice(su * SUBW, (su + 1) * SUBW)
                        pmn = pst.tile([128, SUBW], f32, tag="stat", name="pmn")
                        for hh in range(2):
                            nc.tensor.matmul(pmn, onesC_b, src_pair[hh][:, cs],
                                             start=(hh == 0), stop=(hh == 1))
                        pms = pst.tile([128, SUBW], f32, tag="stat", name="pms")
                        for hh in range(2):
                            nc.tensor.matmul(pms, onesC_b, a2[hh][:, cs],
                                             start=(hh == 0), stop=(hh == 1))
                        nc.scalar.activation(out=mean_sb[0:1, cs], in_=pmn[0:1, :],
                                             func=AF.Copy)
                        m2s = p2.tile([128, SUBW], f32, tag="m2s", bufs=4,
                                      name=f"m2s_{smp_tag}")
                        nc.scalar.activation(out=m2s, in_=pmn, func=AF.Square)
                        nc.vector.tensor_sub(var_[:, cs], pms, m2s)
                    nc.scalar.activation(out=var_, in_=var_, func=AF.Sqrt,
                                         bias=eps_t)
                    rstd = p2.tile([128, CH], f32, tag="rstd", bufs=3,
                                   name=f"rstd_{smp_tag}")
                    nc.vector.reciprocal_approx_fast(out=rstd, in_=var_)
                    return rstd, mean_sb

                def back_stages(cc):
                    g0 = cc * CH_G
                    z = z_store.pop(cc)
                    # S1: att = z @ Wtp + px_unf
                    att = []
                    for mh in range(2):
                        pm = layer_half(z, wtp, mh)
                        ah = p2.tile([128, CH], bf16, tag="att", bufs=4, name="att")
                        nc.vector.scalar_tensor_tensor(
                            out=_view(ah, 0, [[L, CH_G], [1, L]]),
                            in0=_view(pm, 0, [[L, CH_G], [1, L]]),
                            scalar=0.0,
                            in1=unf(px_f[mh], g0, CH_G),
                            op0=ALU.add, op1=ALU.add)
                        att.append(ah)
                    yield
                    # S2: FFN LN stats
                    rstd1, mean1 = ln_stats(att, square_pair(att, f"f{cc}"), f"f{cc}")
                    yield
                    # S3/S4: p1 + rstd apply + gelu, one output half per stage
                    g1 = []
                    for mh in range(2):
                        pm = layer_half(att, w1g, mh, outer_row=(negg[0], mean1))
                        tg = p2.tile([128, CH], bf16, tag="tg", bufs=3, name="tg")
                        nc.vector.scalar_tensor_tensor(
                            out=tg, in0=pm, scalar=1.0, in1=rstd1,
                            op0=ALU.mult, op1=ALU.mult)
                        gh = p2.tile([128, CH], bf16, tag="g1", bufs=3, name="g1")
                        nc.scalar.activation(out=gh, in_=tg, func=AF.Gelu,
                                             bias=B1_t[:, mh:mh + 1])
                        g1.append(gh)
                        yield
                    # S5: p2 + residual
                    y = []
                    for mh in range(2):
                        pm = layer_half(g1, w2b, mh)
                        ysc = p2.tile([128, CH], bf16, tag="ysc", bufs=2, name="ysc")
                        nc.scalar.activation(out=ysc, in_=pm, func=AF.Identity,
                                             bias=bias["b2"][:, mh:mh + 1])
                        yh = p2.tile([128, CH], bf16, tag="ytag", bufs=3, name="y")
                        nc.gpsimd.tensor_add(yh, ysc, att[mh])
                        y.append(yh)
                    yield
                    # S6: temporal LN stats
                    rstd2, mean2 = ln_stats(y, square_pair(y, f"t{cc}"), f"t{cc}")
                    yield
                    # S7/S8: p3 + rstd apply + gelu into w-major layout
                    h_act = []
                    for mh in range(2):
                        pm = layer_half(y, c1g, mh, outer_row=(negg[1], mean2))
                        tg2 = p2.tile([128, CH], bf16, tag="tg", bufs=3, name="tg2")
                        nc.vector.scalar_tensor_tensor(
                            out=tg2, in0=pm, scalar=1.0, in1=rstd2,
                            op0=ALU.mult, op1=ALU.mult)
                        hh_ = p2.tile([128, CH], bf16, tag="hact", bufs=2, name="h_act")
                        dst = _view(hh_, 0, [[V, CH_G], [CH_G * V, W], [1, V]])
                        nc.scalar.activation(out=dst, in_=tg2, func=AF.Gelu,
                                             bias=Bc1_t[:, mh:mh + 1])
                        h_act.append(hh_)
                        yield
                    # S9: c2 contraction (w, i) -> out [O, CH_G*V]
                    for mh in range(2):
                        po = pst.tile([128, CH_G * V], f32, tag="stat", name="po")
                        first = True
                        for w in range(W):
                            for kh in range(2):
                                rhs = h_act[kh][:, w * CH_G * V:(w + 1) * CH_G * V]
                                nc.tensor.matmul(po, c2b[w][kh][:, mh * 128:(mh + 1) * 128],
                                                 rhs, start=first,
                                                 stop=(w == W - 1 and kh == 1))
                                first = False
                        os_ = p2.tile([128, CH_G * V], f32, tag="os", bufs=2, name="os_")
                        nc.scalar.activation(out=os_, in_=po, func=AF.Identity,
                                             bias=bias["c2_b"][:, mh:mh + 1])
                        nc.sync.dma_start(
                            out=out_d[mh * 128:(mh + 1) * 128, g0:g0 + CH_G, :],
                            in_=os_)

                NSTAGE = 9
                START = [0, 2, 4, 6, 8, 10, 12, 14]
                gens = [back_stages(cc) for cc in range(N_CH)]
                front_sched = {}
                for cc in range(2, N_CH):
                    r1a = max(0, START[cc] - 6)
                    front_sched.setdefault(r1a, []).append(lambda c=cc: front_1a(c))
                    front_sched.setdefault(max(r1a + 1, START[cc] - 5), []).append(lambda c=cc: front_1b(c))
                    front_sched.setdefault(max(r1a + 2, START[cc] - 3), []).append(lambda c=cc: front_2(c))
                for r in range(START[-1] + NSTAGE):
                    # back stages first: their psum-consumer ops land early in
                    # the engine FIFOs; latency-tolerant front chains go last
                    for cc in range(N_CH):
                        if 0 <= r - START[cc] < NSTAGE:
                            next(gens[cc], None)
                    for fn in front_sched.get(r, []):
                        fn()

            fps_cm.__exit__(None, None, None)
            frp_cm.__exit__(None, None, None)
    return nc


_CACHE = {}


def _get_compiled():
    if "nc" not in _CACHE:
        nc = bacc.Bacc("TRN2", target_bir_lowering=False, debug=False)
        build(nc)
        nc.compile()
        _CACHE["nc"] = nc
    return _CACHE["nc"]


def kernel(**inputs):
    nc = _get_compiled()
    x = np.asarray(inputs["x"], dtype=np.float32)
    n = x.shape[0]
    names = ["Wq", "Wk", "Wv", "Wt", "Wp", "W1", "W2", "c1_w", "Wqa", "Wka",
             "c2_w", "ln1_g", "ln1_b", "bq", "bk", "bv", "bt", "bp", "ffn_g",
             "ffn_b", "b1", "b2", "tn_g", "tn_b", "c1_b", "c2_b", "bqa", "bka"]
    shared = {nm: np.asarray(inputs[nm], dtype=np.float32) for nm in names}
    in_maps = [{"x": x[i], **shared} for i in range(n)]
    res = bass_utils.run_bass_kernel_spmd(nc, in_maps, core_ids=list(range(n)))
    return np.stack([res.results[i]["out"] for i in range(n)], axis=0)


if __name__ == "__main__":
    nc = bacc.Bacc("TRN2", target_bir_lowering=False, debug=False)
    build(nc)
    nc.compile()
    print("build+compile OK")

